# revision 40
# baseline (speedup 1.0000x reference)
"""LinearAttention kernel for one TRN2 chip (8 NeuronCores), Bass/Tile.

Math (per batch b):
  qkv = x @ w_qkv.T ; q,k,v split, per-head [n, 64]
  k_s = softmax(k, axis=-1)              (over dh, per token/head)
  context_h = k_s^T @ v                  [64, 64]
  out_h = q_h @ context_h ; y = out @ w_out.T + b

Restructured as:
  CT_h = (v/s)-weighted partial: CT[e,d] = sum_n v[n,e]/s[n,h] * exp(k[n,d])
  G_h  = context_h @ w_out_h^T   -> G [inner=512, 1024] block rows
  y    = q @ G + b               (single K=512 matmul)

Sharding: 8 shards = (batch, half-sequence); each core computes its
2048 tokens end-to-end; only the tiny per-batch context (128 KiB) is
all-reduced between the two cores sharing a batch.

Perf structure (v2):
  - warmup matmuls on junk data while the first DMAs land (HAM warm)
  - startup DMAs split across the SP and Pool issue queues
  - PSUM pools for the kv, ct and q phases coexist (2+2+2+2 banks), so
    the Tile scheduler overlaps the q projection with the last kv
    tile's softmax chain; same for G/y vs the q tail
  - ct partial sums combined with an AllReduce pair collective
  - y computed transposed ([feat, tok]) so the bias is a per-partition
    scalar folded into the Scalar-engine PSUM->SBUF copy; output is
    bf16 and the host transposes back
"""

import contextlib
import ctypes
import os
import sys
import types

import numpy as np
import ml_dtypes

# ---------------------------------------------------------------------------
# Compat shim 1: the walrus in this image supports only ONE semaphore wait
# per instruction; split multi-wait instructions into prefix NoOps.
# ---------------------------------------------------------------------------
_MAX_WAITS = 1


def _legalize_bir(bir_bytes: bytes) -> bytes:
    import orjson

    bir = orjson.loads(bir_bytes)
    changed = False
    for fn in bir.get("functions", []):
        for blk in fn.get("blocks", []):
            new_insts = []
            for ins in blk.get("instructions", []):
                si = ins.get("sync_info") or {}
                waits = si.get("on_wait") or []
                if len(waits) > _MAX_WAITS:
                    changed = True
                    extra, keep = waits[:-_MAX_WAITS], waits[-_MAX_WAITS:]
                    for i in range(0, len(extra), _MAX_WAITS):
                        new_insts.append(
                            {
                                "name": f"{ins['name']}-ws{i}",
                                "opcode": "NoOp",
                                "engine": ins["engine"],
                                "ins": [],
                                "outs": [],
                                "sync_info": {
                                    "on_update": [],
                                    "on_wait": extra[i : i + _MAX_WAITS],
                                },
                            }
                        )
                    si["on_wait"] = keep
                new_insts.append(ins)
            blk["instructions"] = new_insts
    if not changed:
        return bir_bytes
    return orjson.dumps(bir)


_compile_patched = False


def _install_compile_patch():
    global _compile_patched
    if _compile_patched:
        return
    import concourse.bass2jax as bass2jax
    import concourse.bass_utils as bass_utils

    orig = bass2jax.compile_bir_kernel

    def compile_bir_kernel_legalized(bir_json, tmpdir, neff_name="file.neff"):
        return orig(_legalize_bir(bytes(bir_json)), tmpdir, neff_name=neff_name)

    bass2jax.compile_bir_kernel = compile_bir_kernel_legalized

    max_sem = os.environ.get("LINATTN_MAX_SEM")
    if os.environ.get("LINATTN_LDW_OPT") or max_sem:
        orig_run = bass_utils.run_command

        def run_command_patched(argv, **kw):
            if os.environ.get("LINATTN_LDW_OPT"):
                argv = [
                    "--enable-ldw-opt=true" if a == "--enable-ldw-opt=false" else a
                    for a in argv
                ]
            if max_sem and any("--neff-output-filename" in str(a) for a in argv):
                argv = list(argv) + [f"--max-sem-num={max_sem}"]
            return orig_run(argv, **kw)

        bass_utils.run_command = run_command_patched
    _compile_patched = True


# ---------------------------------------------------------------------------
# Compat shim 2: NTFF profiling hook (only needed when BASS_TRACE is set).
# ---------------------------------------------------------------------------
def _install_ntff_hook():
    import antenv

    if "antenv.axon_hooks" in sys.modules:
        return
    so_path = "/opt/axon/libaxon_pjrt.so"

    def _mk(so_path):
        try:
            lib = ctypes.CDLL(so_path)
        except OSError:
            return None
        if not hasattr(lib, "axon_start_nrt_profile"):
            return None
        lib.axon_start_nrt_profile.argtypes = [
            ctypes.POINTER(ctypes.c_int64),
            ctypes.c_size_t,
        ]
        lib.axon_start_nrt_profile.restype = ctypes.c_int64
        lib.axon_stop_nrt_profile.argtypes = [ctypes.c_char_p]
        lib.axon_stop_nrt_profile.restype = ctypes.c_int64

        @contextlib.contextmanager
        def _hook(output_dir, device_ids):
            import jax

            jax.devices()
            if device_ids:
                ids = (ctypes.c_int64 * len(device_ids))(*device_ids)
                rc = lib.axon_start_nrt_profile(ids, len(device_ids))
            else:
                rc = lib.axon_start_nrt_profile(None, 0)
            if rc != 0:
                raise RuntimeError(f"axon_start_nrt_profile rc={rc}")
            try:
                yield
            finally:
                n = lib.axon_stop_nrt_profile(str(output_dir).encode())
                if n < 0:
                    raise RuntimeError(f"axon_stop_nrt_profile rc={n}")

        return _hook

    hook = _mk(so_path)
    mod = types.ModuleType("antenv.axon_hooks")
    mod.get_axon_ntff_profile_hook = lambda: hook
    mod.set_axon_ntff_profile_hook = lambda h: None
    sys.modules["antenv.axon_hooks"] = mod
    antenv.axon_hooks = mod


# ---------------------------------------------------------------------------
# Kernel
# ---------------------------------------------------------------------------
B, SEQ, D = 4, 4096, 1024
HEADS, DH = 8, 64
INNER = HEADS * DH  # 512
NCORES = 8
NTOK = B * SEQ // NCORES  # 2048 tokens per core
NT = NTOK // 128  # 16
CK = D // 128  # 8 contraction chunks for the qkv projection
KC = INNER // 128  # 4 contraction chunks for the output projection
FB = D // 128  # 8 output-feature chunks (yT partitions)
REPLICA_GROUPS = [[0, 1], [2, 3], [4, 5], [6, 7]]

_BUILT = None
LAST_RESULT = {}


def build_kernel():
    import concourse.bass as bass
    import concourse.mybir as mybir
    import concourse.tile as tile

    BF = mybir.dt.bfloat16
    F32 = mybir.dt.float32
    EXP = mybir.ActivationFunctionType.Exp
    COPY = mybir.ActivationFunctionType.Copy
    IDENT = mybir.ActivationFunctionType.Identity
    X = mybir.AxisListType.X

    nc = bass.Bass(name="linattn")
    xT = nc.declare_dram_parameter("xT", [D, NTOK], BF, isOutput=False)
    wqT = nc.declare_dram_parameter("wqT", [D, INNER], BF, isOutput=False)
    wkvT = nc.declare_dram_parameter("wkvT", [D, 2 * INNER], BF, isOutput=False)
    woutT = nc.declare_dram_parameter("woutT", [INNER, D], BF, isOutput=False)
    biasT = nc.declare_dram_parameter("biasT", [128, FB], F32, isOutput=False)
    yT = nc.declare_dram_parameter("yT", [D, NTOK], BF, isOutput=True)

    with contextlib.ExitStack() as ctx:
        tc = ctx.enter_context(tile.TileContext(nc))
        cpool = ctx.enter_context(tc.tile_pool(name="const", bufs=1))
        wpool = ctx.enter_context(tc.tile_pool(name="work", bufs=4))
        dpool = ctx.enter_context(tc.tile_pool(name="dram", bufs=1, space="DRAM"))

        # ---- PE warmup: junk matmuls while the first DMAs land ------------
        # HAM un-throttles after ~3.4us of sustained PE activity; these run
        # at 1.2 GHz during the otherwise-idle DMA window so the real
        # matmuls start at 2.4 GHz.
        warm = cpool.tile([128, 512], BF, name="warm")
        nc.vector.memset(warm[:], 0.0)
        ps_warm_cm = tc.tile_pool(name="ps_warm", bufs=1, space="PSUM")
        ps_warm = ps_warm_cm.__enter__()
        w_ps = ps_warm.tile([128, 512], F32, name="w_ps")
        for _ in range(10):
            nc.tensor.matmul(
                w_ps[:], lhsT=warm[:, :128], rhs=warm[:], start=True, stop=True
            )
        ps_warm_cm.__exit__(None, None, None)

        # ---- resident loads ------------------------------------------------
        # Startup-critical loads (wkv + first x columns) go through the SP
        # queue in need order — the kv phase is the long pole and feeds the
        # collective chain, so it must start as early as possible. The
        # later-needed wq/wout/bias and last x quarter ride the Pool queue.
        wkvT_r = wkvT.rearrange("(ck p) f -> p ck f", p=128)
        xT_r = xT.rearrange("(ck p) n -> p ck n", p=128)
        wkv_s = cpool.tile([128, CK, 2 * INNER], BF, name="wkv_s")
        x_s = cpool.tile([128, CK, NTOK], BF, name="x_s")
        wq_s = cpool.tile([128, CK, INNER], BF, name="wq_s")
        wout_s = cpool.tile([128, KC, D], BF, name="wout_s")
        biasT_s = cpool.tile([128, FB], F32, name="biasT_s")
        QCK = CK // 4
        for piece in range(4):
            cks = slice(piece * QCK, (piece + 1) * QCK)
            nc.sync.dma_start(wkv_s[:, cks], wkvT_r[:, cks])
            nc.sync.dma_start(x_s[:, cks, :512], xT_r[:, cks, :512])
        for q4 in range(1, 4):
            sl = slice(q4 * 512, (q4 + 1) * 512)
            nc.sync.dma_start(x_s[:, :, sl], xT_r[:, :, sl])

        wkv_t = [wkv_s[:, ck] for ck in range(CK)]
        x_t = [x_s[:, ck] for ck in range(CK)]

        qT_s = cpool.tile([128, KC, NTOK], BF, name="qT_s")
        g_s = cpool.tile([128, KC, D], BF, name="g_s")
        yT_s = cpool.tile([128, FB, NTOK], BF, name="yT_s")
        ct_f = [cpool.tile([DH, HEADS * DH], BF, name=f"ct_f{i}") for i in range(2)]
        # block-diagonal bf16 form: [e-half, pair, half, d]
        ctw = cpool.tile([128, KC, 2, DH], BF, name="ctw")
        nc.vector.memset(ctw[:], 0.0)

        # ---- phase B: k,v projection + softmax + CT partial ---------------
        # CT[e, h*64+d] = sum_n v[n,e]/s[n,h] * exp(k[n,d])   (this core's n)
        # Two n-halves, each followed by its own AllReduce: the first
        # collective also acts as a rendezvous between the core pair, so the
        # second one (the one on the critical path) runs with minimal skew
        # and hides behind the q projection.
        # ps_q opens first (bottom of the LIFO pool stack) and stays open
        # through phase B, so the scheduler can overlap the q projection
        # with the last kv tile's softmax chain (2+2+4 = 8 PSUM banks).
        ps_q = ctx.enter_context(tc.tile_pool(name="ps_q", bufs=2, space="PSUM"))
        ps_ct_cm = tc.tile_pool(name="ps_ct", bufs=2, space="PSUM")
        ps_ct = ps_ct_cm.__enter__()
        # k and v in separate pools: k_ps frees as soon as the Exp reads it
        # (the DVE's v-side scale runs much later), so the next tiles' k
        # matmuls aren't gated behind the whole softmax chain
        ps_k_cm = tc.tile_pool(name="ps_k", bufs=2, space="PSUM")
        ps_k = ps_k_cm.__enter__()
        ps_v_cm = tc.tile_pool(name="ps_v", bufs=2, space="PSUM")
        ps_v = ps_v_cm.__enter__()
        def q_group(i, ntile):
            q_ps = ps_q.tile([128, 512], F32, name="q_ps")
            tsl = slice(ntile * 512, (ntile + 1) * 512)
            first = None
            for ck in range(CK):
                mm = nc.tensor.matmul(
                    q_ps[:],
                    lhsT=wq_s[:, ck, i * 128 : (i + 1) * 128],
                    rhs=x_t[ck][:, tsl],
                    start=(ck == 0),
                    stop=(ck == CK - 1),
                )
                if first is None:
                    first = mm
            nc.scalar.activation(qT_s[:, i, tsl], q_ps[:], COPY)
            return first

        HNT = NT // 2
        ct_mm = [None] * NT
        ct_first = [None] * NT
        kv_mm = [None] * NT
        # bf16 payload: halves the pair-exchange latency on the critical path
        cin = [dpool.tile([128, KC, DH], BF, name=f"cc_in{i}") for i in range(2)]
        cout = [dpool.tile([2, 128, KC, DH], BF, name=f"cc_out{i}") for i in range(2)]
        for stage in range(2):
            ct_ps = ps_ct.tile([DH, HEADS * DH], F32, name="ct_ps")
            for nt in range(stage * HNT, (stage + 1) * HNT):
                k_ps = ps_k.tile([128, INNER], F32, name="k_ps")
                v_ps = ps_v.tile([128, INNER], F32, name="v_ps")
                nsl = slice(nt * 128, (nt + 1) * 128)
                # K and V matmuls stay interleaved per ck so one LDWEIGHTS
                # (the shared x tile) serves both
                for ck in range(CK):
                    nc.tensor.matmul(
                        k_ps[:],
                        lhsT=x_t[ck][:, nsl],
                        rhs=wkv_t[ck][:, :INNER],
                        start=(ck == 0),
                        stop=(ck == CK - 1),
                    )
                    kv_mm[nt] = nc.tensor.matmul(
                        v_ps[:],
                        lhsT=x_t[ck][:, nsl],
                        rhs=wkv_t[ck][:, INNER:],
                        start=(ck == 0),
                        stop=(ck == CK - 1),
                    )
                expk = wpool.tile([128, INNER], BF, name="expk")
                nc.scalar.activation(expk[:], k_ps[:], EXP)
                ssum = wpool.tile([128, HEADS], F32, name="ssum")
                nc.vector.reduce_sum(
                    ssum[:], expk.rearrange("p (h d) -> p h d", d=DH), axis=X
                )
                rec = wpool.tile([128, HEADS], F32, name="rec")
                nc.vector.reciprocal(rec[:], ssum[:])
                vsc = wpool.tile([128, INNER], BF, name="vsc")
                nc.vector.tensor_tensor(
                    vsc.rearrange("p (h d) -> p h d", d=DH),
                    v_ps.rearrange("p (h d) -> p h d", d=DH),
                    rec[:, :, None].to_broadcast([128, HEADS, DH]),
                    mybir.AluOpType.mult,
                )
                for h in range(HEADS):
                    hs = slice(h * DH, (h + 1) * DH)
                    # start=True clears the whole PSUM *bank*, so only the
                    # very first matmul of the bank may set it; later heads'
                    # first write lands on has_written=0 elems -> overwrite.
                    ct_mm[nt] = nc.tensor.matmul(
                        ct_ps[:, hs],
                        lhsT=vsc[:, hs],
                        rhs=expk[:, hs],
                        start=(nt == stage * HNT and h == 0),
                        stop=(nt == (stage + 1) * HNT - 1),
                        skip_group_check=True,
                    )
                    if h == 0:
                        ct_first[nt] = ct_mm[nt]
            # pack even heads on partitions 0-63, odd heads on 64-127, then
            # pair-AllGather. The PSUM->SBUF copy runs on Scalar (idle at
            # the stage end, unlike the DVE which still holds the last
            # tile's softmax ops), and the packing DMA is a single issue.
            nc.scalar.activation(ct_f[stage][:], ct_ps[:], COPY)
            cin_r = cin[stage].rearrange("(two e) k d -> e k two d", two=2)
            ctf_r = ct_f[stage].rearrange("e (k two d) -> e k two d", two=2, d=DH)
            nc.gpsimd.dma_start(cin_r, ctf_r)
            nc.gpsimd.collective_compute(
                "AllGather",
                mybir.AluOpType.bypass,
                replica_groups=REPLICA_GROUPS,
                ins=[cin[stage].opt()],
                outs=[cout[stage].opt()],
            )
            if stage == 0:
                # queue the later-phase weight loads behind the x pieces
                nc.sync.dma_start(
                    wq_s[:], wqT.rearrange("(ck p) f -> p ck f", p=128)
                )
                nc.sync.dma_start(
                    wout_s[:], woutT.rearrange("(kc p) f -> p kc f", p=128)
                )
                nc.sync.dma_start(biasT_s[:], biasT[:])
        # During the DMA-paced start of B, keep kv matmuls ahead of the
        # softmax-chain-blocked ct matmuls in the static PE order, so a
        # late vsc doesn't stall the queued kv work behind it.
        for nt in range(6):
            tile.add_dep_helper(
                ct_first[nt].ins,
                kv_mm[nt + 1].ins,
                sync=False,
                reason="early ct behind next kv tile",
            )

        ps_v_cm.__exit__(None, None, None)
        ps_k_cm.__exit__(None, None, None)
        ps_ct_cm.__exit__(None, None, None)

        # rank-sum each stage's gathered pair as soon as it lands, then sum
        # the stages and write the block-diagonal bf16 form for the G matmul
        ch = [cpool.tile([128, 2 * KC * DH], BF, name=f"ct_h{i}") for i in range(2)]
        cs = [cpool.tile([128, KC * DH], F32, name=f"ct_s{i}") for i in range(2)]
        for stage in range(2):
            nc.gpsimd.dma_start(
                ch[stage].rearrange("p (r k d) -> p r k d", r=2, d=DH),
                cout[stage].rearrange("r p k d -> p r k d"),
            )
            # on gpsimd: keep the DVE free for the stage-1 softmax tail
            nc.gpsimd.tensor_add(
                cs[stage][:], ch[stage][:, : KC * DH], ch[stage][:, KC * DH :]
            )
        cs_r = [c.rearrange("p (k d) -> p k d", d=DH) for c in cs]
        nc.vector.tensor_add(ctw[:DH, :, 0, :], cs_r[0][:DH], cs_r[1][:DH])
        nc.vector.tensor_add(ctw[DH:, :, 1, :], cs_r[0][DH:], cs_r[1][DH:])

        # ---- phase C: q^T projection (overlaps the collective) ------------
        # Phase B must finish ASAP (the collective chain it feeds is
        # latency-bound), so gate the q matmuls behind the end of B: two
        # groups may start after ct[NT-2] (they cover the last tile's
        # softmax-chain stall), the rest only after the last ct matmul.
        # Without this the scheduler slots q work into every modeled B
        # stall and pushes the collective ~20us later.
        gidx = 0
        for i in range(KC):
            for ntile in range(4):
                first = q_group(i, ntile)
                gate = ct_mm[NT - 2] if gidx < 1 else ct_mm[NT - 1]
                tile.add_dep_helper(
                    first.ins, gate.ins, sync=False, reason="hold q behind phase B"
                )
                gidx += 1

        # ---- phase D: G = blockdiag(context^T) @ w_out^T -------------------
        ps_g = ctx.enter_context(tc.tile_pool(name="ps_g", bufs=2, space="PSUM"))
        ps_y = ctx.enter_context(tc.tile_pool(name="ps_y", bufs=4, space="PSUM"))
        for pr in range(KC):
            lhs = ctw[:, pr].rearrange("p two d -> p (two d)")
            for half in range(2):
                hsl = slice(half * 512, (half + 1) * 512)
                g_ps = ps_g.tile([128, 512], F32, name="g_ps")
                nc.tensor.matmul(
                    g_ps[:], lhsT=lhs, rhs=wout_s[:, pr, hsl], start=True, stop=True
                )
                nc.vector.tensor_copy(g_s[:, pr, hsl], g_ps[:])

        # ---- phase E: yT = G^T @ q^T + b (bias folded into the Scalar copy)
        for f in range(FB):
            fsl = slice(f * 128, (f + 1) * 128)
            for th in range(4):
                tsl = slice(th * 512, (th + 1) * 512)
                y_ps = ps_y.tile([128, 512], F32, name="y_ps")
                for kc in range(KC):
                    nc.tensor.matmul(
                        y_ps[:],
                        lhsT=g_s[:, kc, fsl],
                        rhs=qT_s[:, kc, tsl],
                        start=(kc == 0),
                        stop=(kc == KC - 1),
                    )
                nc.scalar.activation(
                    yT_s[:, f, tsl], y_ps[:], IDENT, bias=biasT_s[:, f : f + 1]
                )
                if th == 1:
                    nc.sync.dma_start(yT[fsl, :1024], yT_s[:, f, :1024])
                elif th == 3:
                    nc.sync.dma_start(yT[fsl, 1024:], yT_s[:, f, 1024:])

    return nc


def _prep_inputs(x, w_qkv, w_out, b_out):
    bf16 = ml_dtypes.bfloat16
    x = np.asarray(x, dtype=np.float32)
    w_qkv = np.asarray(w_qkv, dtype=np.float32)
    w_out = np.asarray(w_out, dtype=np.float32)
    b_out = np.asarray(b_out, dtype=np.float32)

    wqT = np.ascontiguousarray(w_qkv[:INNER].T).astype(bf16)  # [D, 512]
    wkvT = np.ascontiguousarray(w_qkv[INNER:].T).astype(bf16)  # [D, 1024]
    woutT = np.ascontiguousarray(w_out.T).astype(bf16)  # [512, D]
    biasT = np.ascontiguousarray(b_out.reshape(FB, 128).T).astype(np.float32)
    xs = x.reshape(B, 2, NTOK, D)
    in_maps = []
    for c in range(NCORES):
        xT = np.ascontiguousarray(xs[c // 2, c % 2].T).astype(bf16)  # [D, NTOK]
        in_maps.append(
            {"xT": xT, "wqT": wqT, "wkvT": wkvT, "woutT": woutT, "biasT": biasT}
        )
    return in_maps


def kernel(x, w_qkv, w_out, b_out):
    global _BUILT
    _install_compile_patch()
    if os.environ.get("BASS_TRACE"):
        _install_ntff_hook()
    from concourse.bass_utils import run_bass_kernel_spmd

    if _BUILT is None:
        _BUILT = build_kernel()
    nc = _BUILT
    in_maps = _prep_inputs(x, w_qkv, w_out, b_out)
    res = run_bass_kernel_spmd(nc, in_maps, core_ids=list(range(NCORES)))
    LAST_RESULT["exec_time_ns"] = res.exec_time_ns
    LAST_RESULT["profile_json"] = res.profile_json
    out = np.empty((B, 2, NTOK, D), dtype=np.float32)
    for c in range(NCORES):
        out[c // 2, c % 2] = np.asarray(res.results[c]["yT"]).T.astype(np.float32)
    return out.reshape(B, SEQ, D)


# revision 41
# speedup vs baseline: 1.0223x; 1.0223x over previous
"""LinearAttention kernel for one TRN2 chip (8 NeuronCores), Bass/Tile.

Math (per batch b):
  qkv = x @ w_qkv.T ; q,k,v split, per-head [n, 64]
  k_s = softmax(k, axis=-1)              (over dh, per token/head)
  context_h = k_s^T @ v                  [64, 64]
  out_h = q_h @ context_h ; y = out @ w_out.T + b

Restructured as:
  CT_h = (v/s)-weighted partial: CT[e,d] = sum_n v[n,e]/s[n,h] * exp(k[n,d])
  G_h  = context_h @ w_out_h^T   -> G [inner=512, 1024] block rows
  y    = q @ G + b               (single K=512 matmul)

Sharding: 8 shards = (batch, half-sequence); each core computes its
2048 tokens end-to-end; only the tiny per-batch context (128 KiB) is
all-reduced between the two cores sharing a batch.

Perf structure (v2):
  - warmup matmuls on junk data while the first DMAs land (HAM warm)
  - startup DMAs split across the SP and Pool issue queues
  - PSUM pools for the kv, ct and q phases coexist (2+2+2+2 banks), so
    the Tile scheduler overlaps the q projection with the last kv
    tile's softmax chain; same for G/y vs the q tail
  - ct partial sums combined with an AllReduce pair collective
  - y computed transposed ([feat, tok]) so the bias is a per-partition
    scalar folded into the Scalar-engine PSUM->SBUF copy; output is
    bf16 and the host transposes back
"""

import contextlib
import ctypes
import os
import sys
import types

import numpy as np
import ml_dtypes

# ---------------------------------------------------------------------------
# Compat shim 1: the walrus in this image supports only ONE semaphore wait
# per instruction; split multi-wait instructions into prefix NoOps.
# ---------------------------------------------------------------------------
_MAX_WAITS = 1


def _legalize_bir(bir_bytes: bytes) -> bytes:
    import orjson

    bir = orjson.loads(bir_bytes)
    changed = False
    for fn in bir.get("functions", []):
        for blk in fn.get("blocks", []):
            new_insts = []
            for ins in blk.get("instructions", []):
                si = ins.get("sync_info") or {}
                waits = si.get("on_wait") or []
                if len(waits) > _MAX_WAITS:
                    changed = True
                    extra, keep = waits[:-_MAX_WAITS], waits[-_MAX_WAITS:]
                    for i in range(0, len(extra), _MAX_WAITS):
                        new_insts.append(
                            {
                                "name": f"{ins['name']}-ws{i}",
                                "opcode": "NoOp",
                                "engine": ins["engine"],
                                "ins": [],
                                "outs": [],
                                "sync_info": {
                                    "on_update": [],
                                    "on_wait": extra[i : i + _MAX_WAITS],
                                },
                            }
                        )
                    si["on_wait"] = keep
                new_insts.append(ins)
            blk["instructions"] = new_insts
    if not changed:
        return bir_bytes
    return orjson.dumps(bir)


_compile_patched = False


def _install_compile_patch():
    global _compile_patched
    if _compile_patched:
        return
    import concourse.bass2jax as bass2jax
    import concourse.bass_utils as bass_utils

    orig = bass2jax.compile_bir_kernel

    def compile_bir_kernel_legalized(bir_json, tmpdir, neff_name="file.neff"):
        return orig(_legalize_bir(bytes(bir_json)), tmpdir, neff_name=neff_name)

    bass2jax.compile_bir_kernel = compile_bir_kernel_legalized

    max_sem = os.environ.get("LINATTN_MAX_SEM")
    if os.environ.get("LINATTN_LDW_OPT") or max_sem:
        orig_run = bass_utils.run_command

        def run_command_patched(argv, **kw):
            if os.environ.get("LINATTN_LDW_OPT"):
                argv = [
                    "--enable-ldw-opt=true" if a == "--enable-ldw-opt=false" else a
                    for a in argv
                ]
            if max_sem and any("--neff-output-filename" in str(a) for a in argv):
                argv = list(argv) + [f"--max-sem-num={max_sem}"]
            return orig_run(argv, **kw)

        bass_utils.run_command = run_command_patched
    _compile_patched = True


# ---------------------------------------------------------------------------
# Compat shim 2: NTFF profiling hook (only needed when BASS_TRACE is set).
# ---------------------------------------------------------------------------
def _install_ntff_hook():
    import antenv

    if "antenv.axon_hooks" in sys.modules:
        return
    so_path = "/opt/axon/libaxon_pjrt.so"

    def _mk(so_path):
        try:
            lib = ctypes.CDLL(so_path)
        except OSError:
            return None
        if not hasattr(lib, "axon_start_nrt_profile"):
            return None
        lib.axon_start_nrt_profile.argtypes = [
            ctypes.POINTER(ctypes.c_int64),
            ctypes.c_size_t,
        ]
        lib.axon_start_nrt_profile.restype = ctypes.c_int64
        lib.axon_stop_nrt_profile.argtypes = [ctypes.c_char_p]
        lib.axon_stop_nrt_profile.restype = ctypes.c_int64

        @contextlib.contextmanager
        def _hook(output_dir, device_ids):
            import jax

            jax.devices()
            if device_ids:
                ids = (ctypes.c_int64 * len(device_ids))(*device_ids)
                rc = lib.axon_start_nrt_profile(ids, len(device_ids))
            else:
                rc = lib.axon_start_nrt_profile(None, 0)
            if rc != 0:
                raise RuntimeError(f"axon_start_nrt_profile rc={rc}")
            try:
                yield
            finally:
                n = lib.axon_stop_nrt_profile(str(output_dir).encode())
                if n < 0:
                    raise RuntimeError(f"axon_stop_nrt_profile rc={n}")

        return _hook

    hook = _mk(so_path)
    mod = types.ModuleType("antenv.axon_hooks")
    mod.get_axon_ntff_profile_hook = lambda: hook
    mod.set_axon_ntff_profile_hook = lambda h: None
    sys.modules["antenv.axon_hooks"] = mod
    antenv.axon_hooks = mod


# ---------------------------------------------------------------------------
# Kernel
# ---------------------------------------------------------------------------
B, SEQ, D = 4, 4096, 1024
HEADS, DH = 8, 64
INNER = HEADS * DH  # 512
NCORES = 8
NTOK = B * SEQ // NCORES  # 2048 tokens per core
NT = NTOK // 128  # 16
CK = D // 128  # 8 contraction chunks for the qkv projection
KC = INNER // 128  # 4 contraction chunks for the output projection
FB = D // 128  # 8 output-feature chunks (yT partitions)
REPLICA_GROUPS = [[0, 1], [2, 3], [4, 5], [6, 7]]

_BUILT = None
LAST_RESULT = {}


def build_kernel():
    import concourse.bass as bass
    import concourse.mybir as mybir
    import concourse.tile as tile

    BF = mybir.dt.bfloat16
    F32 = mybir.dt.float32
    EXP = mybir.ActivationFunctionType.Exp
    COPY = mybir.ActivationFunctionType.Copy
    IDENT = mybir.ActivationFunctionType.Identity
    X = mybir.AxisListType.X

    nc = bass.Bass(name="linattn")
    xT = nc.declare_dram_parameter("xT", [D, NTOK], BF, isOutput=False)
    wqT = nc.declare_dram_parameter("wqT", [D, INNER], BF, isOutput=False)
    wkvT = nc.declare_dram_parameter("wkvT", [D, 2 * INNER], BF, isOutput=False)
    woutT = nc.declare_dram_parameter("woutT", [INNER, D], BF, isOutput=False)
    biasT = nc.declare_dram_parameter("biasT", [128, FB], F32, isOutput=False)
    yT = nc.declare_dram_parameter("yT", [D, NTOK], BF, isOutput=True)

    with contextlib.ExitStack() as ctx:
        tc = ctx.enter_context(tile.TileContext(nc))
        cpool = ctx.enter_context(tc.tile_pool(name="const", bufs=1))
        wpool = ctx.enter_context(tc.tile_pool(name="work", bufs=4))
        dpool = ctx.enter_context(tc.tile_pool(name="dram", bufs=1, space="DRAM"))

        # ---- PE warmup: junk matmuls while the first DMAs land ------------
        # HAM un-throttles after ~3.4us of sustained PE activity; these run
        # at 1.2 GHz during the otherwise-idle DMA window so the real
        # matmuls start at 2.4 GHz.
        warm = cpool.tile([128, 512], BF, name="warm")
        nc.vector.memset(warm[:], 0.0)
        ps_warm_cm = tc.tile_pool(name="ps_warm", bufs=1, space="PSUM")
        ps_warm = ps_warm_cm.__enter__()
        w_ps = ps_warm.tile([128, 512], F32, name="w_ps")
        for _ in range(10):
            nc.tensor.matmul(
                w_ps[:], lhsT=warm[:, :128], rhs=warm[:], start=True, stop=True
            )
        ps_warm_cm.__exit__(None, None, None)

        # ---- resident loads ------------------------------------------------
        # Startup-critical loads (wkv + first x columns) go through the SP
        # queue in need order — the kv phase is the long pole and feeds the
        # collective chain, so it must start as early as possible. The
        # later-needed wq/wout/bias and last x quarter ride the Pool queue.
        wkvT_r = wkvT.rearrange("(ck p) f -> p ck f", p=128)
        xT_r = xT.rearrange("(ck p) n -> p ck n", p=128)
        wkv_s = cpool.tile([128, CK, 2 * INNER], BF, name="wkv_s")
        x_s = cpool.tile([128, CK, NTOK], BF, name="x_s")
        wq_s = cpool.tile([128, CK, INNER], BF, name="wq_s")
        wout_s = cpool.tile([128, KC, D], BF, name="wout_s")
        biasT_s = cpool.tile([128, FB], F32, name="biasT_s")
        QCK = CK // 4
        for piece in range(4):
            cks = slice(piece * QCK, (piece + 1) * QCK)
            nc.sync.dma_start(wkv_s[:, cks], wkvT_r[:, cks])
            nc.sync.dma_start(x_s[:, cks, :512], xT_r[:, cks, :512])
        for q4 in range(1, 4):
            sl = slice(q4 * 512, (q4 + 1) * 512)
            nc.sync.dma_start(x_s[:, :, sl], xT_r[:, :, sl])

        wkv_t = [wkv_s[:, ck] for ck in range(CK)]
        x_t = [x_s[:, ck] for ck in range(CK)]

        qT_s = cpool.tile([128, KC, NTOK], BF, name="qT_s")
        g_s = cpool.tile([128, KC, D], BF, name="g_s")
        yT_s = cpool.tile([128, FB, NTOK], BF, name="yT_s")
        ct_f = [cpool.tile([DH, HEADS * DH], BF, name=f"ct_f{i}") for i in range(2)]
        # block-diagonal bf16 form: [e-half, pair, half, d]
        ctw = cpool.tile([128, KC, 2, DH], BF, name="ctw")
        nc.vector.memset(ctw[:], 0.0)

        # ---- phase B: k,v projection + softmax + CT partial ---------------
        # CT[e, h*64+d] = sum_n v[n,e]/s[n,h] * exp(k[n,d])   (this core's n)
        # Two n-halves, each followed by its own AllReduce: the first
        # collective also acts as a rendezvous between the core pair, so the
        # second one (the one on the critical path) runs with minimal skew
        # and hides behind the q projection.
        # ps_q opens first (bottom of the LIFO pool stack) and stays open
        # through phase B, so the scheduler can overlap the q projection
        # with the last kv tile's softmax chain (2+2+4 = 8 PSUM banks).
        ps_q = ctx.enter_context(tc.tile_pool(name="ps_q", bufs=2, space="PSUM"))
        ps_ct_cm = tc.tile_pool(name="ps_ct", bufs=2, space="PSUM")
        ps_ct = ps_ct_cm.__enter__()
        # k and v in separate pools: k_ps frees as soon as the Exp reads it
        # (the DVE's v-side scale runs much later), so the next tiles' k
        # matmuls aren't gated behind the whole softmax chain
        ps_k_cm = tc.tile_pool(name="ps_k", bufs=2, space="PSUM")
        ps_k = ps_k_cm.__enter__()
        ps_v_cm = tc.tile_pool(name="ps_v", bufs=2, space="PSUM")
        ps_v = ps_v_cm.__enter__()
        def q_group(i, ntile):
            q_ps = ps_q.tile([128, 512], F32, name="q_ps")
            tsl = slice(ntile * 512, (ntile + 1) * 512)
            first = None
            for ck in range(CK):
                mm = nc.tensor.matmul(
                    q_ps[:],
                    lhsT=wq_s[:, ck, i * 128 : (i + 1) * 128],
                    rhs=x_t[ck][:, tsl],
                    start=(ck == 0),
                    stop=(ck == CK - 1),
                )
                if first is None:
                    first = mm
            nc.scalar.activation(qT_s[:, i, tsl], q_ps[:], COPY)
            return first

        HNT = NT // 2
        ct_mm = [None] * NT
        ct_first = [None] * NT
        kv_mm = [None] * NT
        # bf16 payload: halves the pair-exchange latency on the critical path
        cin = [dpool.tile([128, KC, DH], BF, name=f"cc_in{i}") for i in range(2)]
        cout = [dpool.tile([2, 128, KC, DH], BF, name=f"cc_out{i}") for i in range(2)]
        for stage in range(2):
            ct_ps = ps_ct.tile([DH, HEADS * DH], F32, name="ct_ps")
            for nt in range(stage * HNT, (stage + 1) * HNT):
                k_ps = ps_k.tile([128, INNER], F32, name="k_ps")
                v_ps = ps_v.tile([128, INNER], F32, name="v_ps")
                nsl = slice(nt * 128, (nt + 1) * 128)
                # K and V matmuls stay interleaved per ck so one LDWEIGHTS
                # (the shared x tile) serves both
                for ck in range(CK):
                    nc.tensor.matmul(
                        k_ps[:],
                        lhsT=x_t[ck][:, nsl],
                        rhs=wkv_t[ck][:, :INNER],
                        start=(ck == 0),
                        stop=(ck == CK - 1),
                    )
                    kv_mm[nt] = nc.tensor.matmul(
                        v_ps[:],
                        lhsT=x_t[ck][:, nsl],
                        rhs=wkv_t[ck][:, INNER:],
                        start=(ck == 0),
                        stop=(ck == CK - 1),
                    )
                expk = wpool.tile([128, INNER], BF, name="expk")
                nc.scalar.activation(expk[:], k_ps[:], EXP)
                ssum = wpool.tile([128, HEADS], F32, name="ssum")
                nc.vector.reduce_sum(
                    ssum[:], expk.rearrange("p (h d) -> p h d", d=DH), axis=X
                )
                rec = wpool.tile([128, HEADS], F32, name="rec")
                nc.vector.reciprocal(rec[:], ssum[:])
                vsc = wpool.tile([128, INNER], BF, name="vsc")
                nc.vector.tensor_tensor(
                    vsc.rearrange("p (h d) -> p h d", d=DH),
                    v_ps.rearrange("p (h d) -> p h d", d=DH),
                    rec[:, :, None].to_broadcast([128, HEADS, DH]),
                    mybir.AluOpType.mult,
                )
                for h in range(HEADS):
                    hs = slice(h * DH, (h + 1) * DH)
                    # start=True clears the whole PSUM *bank*, so only the
                    # very first matmul of the bank may set it; later heads'
                    # first write lands on has_written=0 elems -> overwrite.
                    ct_mm[nt] = nc.tensor.matmul(
                        ct_ps[:, hs],
                        lhsT=vsc[:, hs],
                        rhs=expk[:, hs],
                        start=(nt == stage * HNT and h == 0),
                        stop=(nt == (stage + 1) * HNT - 1),
                        skip_group_check=True,
                    )
                    if h == 0:
                        ct_first[nt] = ct_mm[nt]
            # pack even heads on partitions 0-63, odd heads on 64-127, then
            # pair-AllGather. The PSUM->SBUF copy runs on Scalar (idle at
            # the stage end, unlike the DVE which still holds the last
            # tile's softmax ops), and the packing DMA is a single issue.
            nc.scalar.activation(ct_f[stage][:], ct_ps[:], COPY)
            cin_r = cin[stage].rearrange("(two e) k d -> e k two d", two=2)
            ctf_r = ct_f[stage].rearrange("e (k two d) -> e k two d", two=2, d=DH)
            nc.gpsimd.dma_start(cin_r, ctf_r)
            nc.gpsimd.collective_compute(
                "AllGather",
                mybir.AluOpType.bypass,
                replica_groups=REPLICA_GROUPS,
                ins=[cin[stage].opt()],
                outs=[cout[stage].opt()],
            )
            if stage == 0:
                # queue the later-phase weight loads behind the x pieces
                nc.sync.dma_start(
                    wq_s[:], wqT.rearrange("(ck p) f -> p ck f", p=128)
                )
                nc.sync.dma_start(
                    wout_s[:], woutT.rearrange("(kc p) f -> p kc f", p=128)
                )
                nc.sync.dma_start(biasT_s[:], biasT[:])
        # During the DMA-paced start of B, keep kv matmuls ahead of the
        # softmax-chain-blocked ct matmuls in the static PE order, so a
        # late vsc doesn't stall the queued kv work behind it.
        for nt in range(6):
            tile.add_dep_helper(
                ct_first[nt].ins,
                kv_mm[nt + 1].ins,
                sync=False,
                reason="early ct behind next kv tile",
            )

        ps_v_cm.__exit__(None, None, None)
        ps_k_cm.__exit__(None, None, None)
        ps_ct_cm.__exit__(None, None, None)

        # rank-sum each stage's gathered pair as soon as it lands, then sum
        # the stages and write the block-diagonal bf16 form for the G matmul
        ch = [cpool.tile([128, 2 * KC * DH], BF, name=f"ct_h{i}") for i in range(2)]
        cs = [cpool.tile([128, KC * DH], F32, name=f"ct_s{i}") for i in range(2)]
        for stage in range(2):
            nc.gpsimd.dma_start(
                ch[stage].rearrange("p (r k d) -> p r k d", r=2, d=DH),
                cout[stage].rearrange("r p k d -> p r k d"),
            )
            # on gpsimd: keep the DVE free for the stage-1 softmax tail
            nc.gpsimd.tensor_add(
                cs[stage][:], ch[stage][:, : KC * DH], ch[stage][:, KC * DH :]
            )
        cs_r = [c.rearrange("p (k d) -> p k d", d=DH) for c in cs]
        nc.vector.tensor_add(ctw[:DH, :, 0, :], cs_r[0][:DH], cs_r[1][:DH])
        nc.vector.tensor_add(ctw[DH:, :, 1, :], cs_r[0][DH:], cs_r[1][DH:])

        # ---- phase C: q^T projection (overlaps the collective) ------------
        # Phase B must finish ASAP (the collective chain it feeds is
        # latency-bound), so gate the q matmuls behind the end of B: two
        # groups may start after ct[NT-2] (they cover the last tile's
        # softmax-chain stall), the rest only after the last ct matmul.
        # Without this the scheduler slots q work into every modeled B
        # stall and pushes the collective ~20us later.
        gidx = 0
        for i in range(KC):
            for ntile in range(4):
                first = q_group(i, ntile)
                gate = ct_mm[NT - 2] if gidx < 2 else ct_mm[NT - 1]
                tile.add_dep_helper(
                    first.ins, gate.ins, sync=False, reason="hold q behind phase B"
                )
                gidx += 1

        # ---- phase D: G = blockdiag(context^T) @ w_out^T -------------------
        ps_g = ctx.enter_context(tc.tile_pool(name="ps_g", bufs=2, space="PSUM"))
        ps_y = ctx.enter_context(tc.tile_pool(name="ps_y", bufs=4, space="PSUM"))
        for pr in range(KC):
            lhs = ctw[:, pr].rearrange("p two d -> p (two d)")
            for half in range(2):
                hsl = slice(half * 512, (half + 1) * 512)
                g_ps = ps_g.tile([128, 512], F32, name="g_ps")
                nc.tensor.matmul(
                    g_ps[:], lhsT=lhs, rhs=wout_s[:, pr, hsl], start=True, stop=True
                )
                nc.vector.tensor_copy(g_s[:, pr, hsl], g_ps[:])

        # ---- phase E: yT = G^T @ q^T + b (bias folded into the Scalar copy)
        for f in range(FB):
            fsl = slice(f * 128, (f + 1) * 128)
            for th in range(4):
                tsl = slice(th * 512, (th + 1) * 512)
                y_ps = ps_y.tile([128, 512], F32, name="y_ps")
                for kc in range(KC):
                    nc.tensor.matmul(
                        y_ps[:],
                        lhsT=g_s[:, kc, fsl],
                        rhs=qT_s[:, kc, tsl],
                        start=(kc == 0),
                        stop=(kc == KC - 1),
                    )
                nc.scalar.activation(
                    yT_s[:, f, tsl], y_ps[:], IDENT, bias=biasT_s[:, f : f + 1]
                )
                nc.sync.dma_start(yT[fsl, tsl], yT_s[:, f, tsl])

    return nc


def _prep_inputs(x, w_qkv, w_out, b_out):
    bf16 = ml_dtypes.bfloat16
    x = np.asarray(x, dtype=np.float32)
    w_qkv = np.asarray(w_qkv, dtype=np.float32)
    w_out = np.asarray(w_out, dtype=np.float32)
    b_out = np.asarray(b_out, dtype=np.float32)

    wqT = np.ascontiguousarray(w_qkv[:INNER].T).astype(bf16)  # [D, 512]
    wkvT = np.ascontiguousarray(w_qkv[INNER:].T).astype(bf16)  # [D, 1024]
    woutT = np.ascontiguousarray(w_out.T).astype(bf16)  # [512, D]
    biasT = np.ascontiguousarray(b_out.reshape(FB, 128).T).astype(np.float32)
    xs = x.reshape(B, 2, NTOK, D)
    in_maps = []
    for c in range(NCORES):
        xT = np.ascontiguousarray(xs[c // 2, c % 2].T).astype(bf16)  # [D, NTOK]
        in_maps.append(
            {"xT": xT, "wqT": wqT, "wkvT": wkvT, "woutT": woutT, "biasT": biasT}
        )
    return in_maps


def kernel(x, w_qkv, w_out, b_out):
    global _BUILT
    _install_compile_patch()
    if os.environ.get("BASS_TRACE"):
        _install_ntff_hook()
    from concourse.bass_utils import run_bass_kernel_spmd

    if _BUILT is None:
        _BUILT = build_kernel()
    nc = _BUILT
    in_maps = _prep_inputs(x, w_qkv, w_out, b_out)
    res = run_bass_kernel_spmd(nc, in_maps, core_ids=list(range(NCORES)))
    LAST_RESULT["exec_time_ns"] = res.exec_time_ns
    LAST_RESULT["profile_json"] = res.profile_json
    out = np.empty((B, 2, NTOK, D), dtype=np.float32)
    for c in range(NCORES):
        out[c // 2, c % 2] = np.asarray(res.results[c]["yT"]).T.astype(np.float32)
    return out.reshape(B, SEQ, D)


# revision 42
# speedup vs baseline: 1.0280x; 1.0056x over previous
"""LinearAttention kernel for one TRN2 chip (8 NeuronCores), Bass/Tile.

Math (per batch b):
  qkv = x @ w_qkv.T ; q,k,v split, per-head [n, 64]
  k_s = softmax(k, axis=-1)              (over dh, per token/head)
  context_h = k_s^T @ v                  [64, 64]
  out_h = q_h @ context_h ; y = out @ w_out.T + b

Restructured as:
  CT_h = (v/s)-weighted partial: CT[e,d] = sum_n v[n,e]/s[n,h] * exp(k[n,d])
  G_h  = context_h @ w_out_h^T   -> G [inner=512, 1024] block rows
  y    = q @ G + b               (single K=512 matmul)

Sharding: 8 shards = (batch, half-sequence); each core computes its
2048 tokens end-to-end; only the tiny per-batch context (128 KiB) is
all-reduced between the two cores sharing a batch.

Perf structure:
  - warmup matmuls on junk data while the first DMAs land (HAM warm)
  - startup-critical DMAs (wkv + first x columns) interleaved finely on
    the SP queue in need order; the Pool queue is reserved for the
    collective-path transfers
  - PSUM pools: separate k/v pools (k_ps frees right after the Exp so
    the next tiles' matmuls aren't gated on the whole softmax chain) +
    ct + q coexist (2+2+1+1+2 banks); q matmuls are explicitly gated
    behind the end of phase B so the latency-bound collective chain
    starts as early as possible, with two groups left free to cover
    the last tile's softmax stall
  - ct partial sums exchanged via bf16 pair-AllGather + on-chip sums
  - y computed transposed ([feat, tok]) so the bias is a per-partition
    scalar folded into the Scalar-engine PSUM->SBUF copy; output is
    bf16 and the host transposes back
"""

import contextlib
import ctypes
import os
import sys
import types

import numpy as np
import ml_dtypes

# ---------------------------------------------------------------------------
# Compat shim 1: the walrus in this image supports only ONE semaphore wait
# per instruction; split multi-wait instructions into prefix NoOps.
# ---------------------------------------------------------------------------
_MAX_WAITS = 1


def _legalize_bir(bir_bytes: bytes) -> bytes:
    import orjson

    bir = orjson.loads(bir_bytes)
    changed = False
    for fn in bir.get("functions", []):
        for blk in fn.get("blocks", []):
            new_insts = []
            for ins in blk.get("instructions", []):
                si = ins.get("sync_info") or {}
                waits = si.get("on_wait") or []
                if len(waits) > _MAX_WAITS:
                    changed = True
                    extra, keep = waits[:-_MAX_WAITS], waits[-_MAX_WAITS:]
                    for i in range(0, len(extra), _MAX_WAITS):
                        new_insts.append(
                            {
                                "name": f"{ins['name']}-ws{i}",
                                "opcode": "NoOp",
                                "engine": ins["engine"],
                                "ins": [],
                                "outs": [],
                                "sync_info": {
                                    "on_update": [],
                                    "on_wait": extra[i : i + _MAX_WAITS],
                                },
                            }
                        )
                    si["on_wait"] = keep
                new_insts.append(ins)
            blk["instructions"] = new_insts
    if not changed:
        return bir_bytes
    return orjson.dumps(bir)


_compile_patched = False


def _install_compile_patch():
    global _compile_patched
    if _compile_patched:
        return
    import concourse.bass2jax as bass2jax
    import concourse.bass_utils as bass_utils

    orig = bass2jax.compile_bir_kernel

    def compile_bir_kernel_legalized(bir_json, tmpdir, neff_name="file.neff"):
        return orig(_legalize_bir(bytes(bir_json)), tmpdir, neff_name=neff_name)

    bass2jax.compile_bir_kernel = compile_bir_kernel_legalized

    max_sem = os.environ.get("LINATTN_MAX_SEM")
    if os.environ.get("LINATTN_LDW_OPT") or max_sem:
        orig_run = bass_utils.run_command

        def run_command_patched(argv, **kw):
            if os.environ.get("LINATTN_LDW_OPT"):
                argv = [
                    "--enable-ldw-opt=true" if a == "--enable-ldw-opt=false" else a
                    for a in argv
                ]
            if max_sem and any("--neff-output-filename" in str(a) for a in argv):
                argv = list(argv) + [f"--max-sem-num={max_sem}"]
            return orig_run(argv, **kw)

        bass_utils.run_command = run_command_patched
    _compile_patched = True


# ---------------------------------------------------------------------------
# Compat shim 2: NTFF profiling hook (only needed when BASS_TRACE is set).
# ---------------------------------------------------------------------------
def _install_ntff_hook():
    import antenv

    if "antenv.axon_hooks" in sys.modules:
        return
    so_path = "/opt/axon/libaxon_pjrt.so"

    def _mk(so_path):
        try:
            lib = ctypes.CDLL(so_path)
        except OSError:
            return None
        if not hasattr(lib, "axon_start_nrt_profile"):
            return None
        lib.axon_start_nrt_profile.argtypes = [
            ctypes.POINTER(ctypes.c_int64),
            ctypes.c_size_t,
        ]
        lib.axon_start_nrt_profile.restype = ctypes.c_int64
        lib.axon_stop_nrt_profile.argtypes = [ctypes.c_char_p]
        lib.axon_stop_nrt_profile.restype = ctypes.c_int64

        @contextlib.contextmanager
        def _hook(output_dir, device_ids):
            import jax

            jax.devices()
            if device_ids:
                ids = (ctypes.c_int64 * len(device_ids))(*device_ids)
                rc = lib.axon_start_nrt_profile(ids, len(device_ids))
            else:
                rc = lib.axon_start_nrt_profile(None, 0)
            if rc != 0:
                raise RuntimeError(f"axon_start_nrt_profile rc={rc}")
            try:
                yield
            finally:
                n = lib.axon_stop_nrt_profile(str(output_dir).encode())
                if n < 0:
                    raise RuntimeError(f"axon_stop_nrt_profile rc={n}")

        return _hook

    hook = _mk(so_path)
    mod = types.ModuleType("antenv.axon_hooks")
    mod.get_axon_ntff_profile_hook = lambda: hook
    mod.set_axon_ntff_profile_hook = lambda h: None
    sys.modules["antenv.axon_hooks"] = mod
    antenv.axon_hooks = mod


# ---------------------------------------------------------------------------
# Kernel
# ---------------------------------------------------------------------------
B, SEQ, D = 4, 4096, 1024
HEADS, DH = 8, 64
INNER = HEADS * DH  # 512
NCORES = 8
NTOK = B * SEQ // NCORES  # 2048 tokens per core
NT = NTOK // 128  # 16
CK = D // 128  # 8 contraction chunks for the qkv projection
KC = INNER // 128  # 4 contraction chunks for the output projection
FB = D // 128  # 8 output-feature chunks (yT partitions)
REPLICA_GROUPS = [[0, 1], [2, 3], [4, 5], [6, 7]]

_BUILT = None
LAST_RESULT = {}


def build_kernel():
    import concourse.bass as bass
    import concourse.mybir as mybir
    import concourse.tile as tile

    BF = mybir.dt.bfloat16
    F32 = mybir.dt.float32
    EXP = mybir.ActivationFunctionType.Exp
    COPY = mybir.ActivationFunctionType.Copy
    IDENT = mybir.ActivationFunctionType.Identity
    X = mybir.AxisListType.X

    nc = bass.Bass(name="linattn")
    xT = nc.declare_dram_parameter("xT", [D, NTOK], BF, isOutput=False)
    wqT = nc.declare_dram_parameter("wqT", [D, INNER], BF, isOutput=False)
    wkvT = nc.declare_dram_parameter("wkvT", [D, 2 * INNER], BF, isOutput=False)
    woutT = nc.declare_dram_parameter("woutT", [INNER, D], BF, isOutput=False)
    biasT = nc.declare_dram_parameter("biasT", [128, FB], F32, isOutput=False)
    yT = nc.declare_dram_parameter("yT", [D, NTOK], BF, isOutput=True)

    with contextlib.ExitStack() as ctx:
        tc = ctx.enter_context(tile.TileContext(nc))
        cpool = ctx.enter_context(tc.tile_pool(name="const", bufs=1))
        wpool = ctx.enter_context(tc.tile_pool(name="work", bufs=4))
        dpool = ctx.enter_context(tc.tile_pool(name="dram", bufs=1, space="DRAM"))

        # ---- PE warmup: junk matmuls while the first DMAs land ------------
        # HAM un-throttles after ~3.4us of sustained PE activity; these run
        # at 1.2 GHz during the otherwise-idle DMA window so the real
        # matmuls start at 2.4 GHz.
        warm = cpool.tile([128, 512], BF, name="warm")
        nc.vector.memset(warm[:], 0.0)
        ps_warm_cm = tc.tile_pool(name="ps_warm", bufs=1, space="PSUM")
        ps_warm = ps_warm_cm.__enter__()
        w_ps = ps_warm.tile([128, 512], F32, name="w_ps")
        for _ in range(10):
            nc.tensor.matmul(
                w_ps[:], lhsT=warm[:, :128], rhs=warm[:], start=True, stop=True
            )
        ps_warm_cm.__exit__(None, None, None)

        # ---- resident loads ------------------------------------------------
        # Startup-critical loads (wkv + first x columns) go through the SP
        # queue in need order — the kv phase is the long pole and feeds the
        # collective chain, so it must start as early as possible. The
        # later-needed wq/wout/bias and last x quarter ride the Pool queue.
        wkvT_r = wkvT.rearrange("(ck p) f -> p ck f", p=128)
        xT_r = xT.rearrange("(ck p) n -> p ck n", p=128)
        wkv_s = cpool.tile([128, CK, 2 * INNER], BF, name="wkv_s")
        x_s = cpool.tile([128, CK, NTOK], BF, name="x_s")
        wq_s = cpool.tile([128, CK, INNER], BF, name="wq_s")
        wout_s = cpool.tile([128, KC, D], BF, name="wout_s")
        biasT_s = cpool.tile([128, FB], F32, name="biasT_s")
        QCK = CK // 4
        for piece in range(4):
            cks = slice(piece * QCK, (piece + 1) * QCK)
            nc.sync.dma_start(wkv_s[:, cks], wkvT_r[:, cks])
            nc.sync.dma_start(x_s[:, cks, :512], xT_r[:, cks, :512])
        for q4 in range(1, 4):
            sl = slice(q4 * 512, (q4 + 1) * 512)
            nc.sync.dma_start(x_s[:, :, sl], xT_r[:, :, sl])

        wkv_t = [wkv_s[:, ck] for ck in range(CK)]
        x_t = [x_s[:, ck] for ck in range(CK)]

        qT_s = cpool.tile([128, KC, NTOK], BF, name="qT_s")
        g_s = cpool.tile([128, KC, D], BF, name="g_s")
        yT_s = cpool.tile([128, FB, NTOK], BF, name="yT_s")
        ct_f = [cpool.tile([DH, HEADS * DH], BF, name=f"ct_f{i}") for i in range(2)]
        # block-diagonal bf16 form: [e-half, pair, half, d]
        ctw = cpool.tile([128, KC, 2, DH], BF, name="ctw")
        nc.vector.memset(ctw[:], 0.0)

        # ---- phase B: k,v projection + softmax + CT partial ---------------
        # CT[e, h*64+d] = sum_n v[n,e]/s[n,h] * exp(k[n,d])   (this core's n)
        # Two n-halves, each followed by its own AllReduce: the first
        # collective also acts as a rendezvous between the core pair, so the
        # second one (the one on the critical path) runs with minimal skew
        # and hides behind the q projection.
        # ps_q opens first (bottom of the LIFO pool stack) and stays open
        # through phase B, so the scheduler can overlap the q projection
        # with the last kv tile's softmax chain (2+2+4 = 8 PSUM banks).
        ps_q = ctx.enter_context(tc.tile_pool(name="ps_q", bufs=2, space="PSUM"))
        ps_ct_cm = tc.tile_pool(name="ps_ct", bufs=2, space="PSUM")
        ps_ct = ps_ct_cm.__enter__()
        # k and v in separate pools: k_ps frees as soon as the Exp reads it
        # (the DVE's v-side scale runs much later), so the next tiles' k
        # matmuls aren't gated behind the whole softmax chain
        ps_k_cm = tc.tile_pool(name="ps_k", bufs=2, space="PSUM")
        ps_k = ps_k_cm.__enter__()
        ps_v_cm = tc.tile_pool(name="ps_v", bufs=2, space="PSUM")
        ps_v = ps_v_cm.__enter__()
        def q_group(i, ntile):
            q_ps = ps_q.tile([128, 512], F32, name="q_ps")
            tsl = slice(ntile * 512, (ntile + 1) * 512)
            first = None
            for ck in range(CK):
                mm = nc.tensor.matmul(
                    q_ps[:],
                    lhsT=wq_s[:, ck, i * 128 : (i + 1) * 128],
                    rhs=x_t[ck][:, tsl],
                    start=(ck == 0),
                    stop=(ck == CK - 1),
                )
                if first is None:
                    first = mm
            nc.scalar.activation(qT_s[:, i, tsl], q_ps[:], COPY)
            return first

        HNT = NT // 2
        ct_mm = [None] * NT
        ct_first = [None] * NT
        kv_mm = [None] * NT
        # bf16 payload: halves the pair-exchange latency on the critical path
        cin = [dpool.tile([128, KC, DH], BF, name=f"cc_in{i}") for i in range(2)]
        cout = [dpool.tile([2, 128, KC, DH], BF, name=f"cc_out{i}") for i in range(2)]
        for stage in range(2):
            ct_ps = ps_ct.tile([DH, HEADS * DH], F32, name="ct_ps")
            for nt in range(stage * HNT, (stage + 1) * HNT):
                k_ps = ps_k.tile([128, INNER], F32, name="k_ps")
                v_ps = ps_v.tile([128, INNER], F32, name="v_ps")
                nsl = slice(nt * 128, (nt + 1) * 128)
                # K and V matmuls stay interleaved per ck so one LDWEIGHTS
                # (the shared x tile) serves both
                for ck in range(CK):
                    nc.tensor.matmul(
                        k_ps[:],
                        lhsT=x_t[ck][:, nsl],
                        rhs=wkv_t[ck][:, :INNER],
                        start=(ck == 0),
                        stop=(ck == CK - 1),
                    )
                    kv_mm[nt] = nc.tensor.matmul(
                        v_ps[:],
                        lhsT=x_t[ck][:, nsl],
                        rhs=wkv_t[ck][:, INNER:],
                        start=(ck == 0),
                        stop=(ck == CK - 1),
                    )
                expk = wpool.tile([128, INNER], BF, name="expk")
                nc.scalar.activation(expk[:], k_ps[:], EXP)
                ssum = wpool.tile([128, HEADS], F32, name="ssum")
                nc.vector.reduce_sum(
                    ssum[:], expk.rearrange("p (h d) -> p h d", d=DH), axis=X
                )
                rec = wpool.tile([128, HEADS], F32, name="rec")
                nc.vector.reciprocal(rec[:], ssum[:])
                vsc = wpool.tile([128, INNER], BF, name="vsc")
                nc.vector.tensor_tensor(
                    vsc.rearrange("p (h d) -> p h d", d=DH),
                    v_ps.rearrange("p (h d) -> p h d", d=DH),
                    rec[:, :, None].to_broadcast([128, HEADS, DH]),
                    mybir.AluOpType.mult,
                )
                for h in range(HEADS):
                    hs = slice(h * DH, (h + 1) * DH)
                    # start=True clears the whole PSUM *bank*, so only the
                    # very first matmul of the bank may set it; later heads'
                    # first write lands on has_written=0 elems -> overwrite.
                    ct_mm[nt] = nc.tensor.matmul(
                        ct_ps[:, hs],
                        lhsT=vsc[:, hs],
                        rhs=expk[:, hs],
                        start=(nt == stage * HNT and h == 0),
                        stop=(nt == (stage + 1) * HNT - 1),
                        skip_group_check=True,
                    )
                    if h == 0:
                        ct_first[nt] = ct_mm[nt]
            # pack even heads on partitions 0-63, odd heads on 64-127, then
            # pair-AllGather. The PSUM->SBUF copy runs on Scalar (idle at
            # the stage end, unlike the DVE which still holds the last
            # tile's softmax ops), and the packing DMA is a single issue.
            nc.scalar.activation(ct_f[stage][:], ct_ps[:], COPY)
            cin_r = cin[stage].rearrange("(two e) k d -> e k two d", two=2)
            ctf_r = ct_f[stage].rearrange("e (k two d) -> e k two d", two=2, d=DH)
            nc.gpsimd.dma_start(cin_r, ctf_r)
            nc.gpsimd.collective_compute(
                "AllGather",
                mybir.AluOpType.bypass,
                replica_groups=REPLICA_GROUPS,
                ins=[cin[stage].opt()],
                outs=[cout[stage].opt()],
            )
            if stage == 0:
                # queue the later-phase weight loads behind the x pieces
                nc.sync.dma_start(
                    wq_s[:], wqT.rearrange("(ck p) f -> p ck f", p=128)
                )
                nc.sync.dma_start(
                    wout_s[:], woutT.rearrange("(kc p) f -> p kc f", p=128)
                )
                nc.sync.dma_start(biasT_s[:], biasT[:])
        # During the DMA-paced start of B, keep kv matmuls ahead of the
        # softmax-chain-blocked ct matmuls in the static PE order, so a
        # late vsc doesn't stall the queued kv work behind it.
        for nt in range(6):
            tile.add_dep_helper(
                ct_first[nt].ins,
                kv_mm[nt + 1].ins,
                sync=False,
                reason="early ct behind next kv tile",
            )

        ps_v_cm.__exit__(None, None, None)
        ps_k_cm.__exit__(None, None, None)
        ps_ct_cm.__exit__(None, None, None)

        # rank-sum each stage's gathered pair as soon as it lands, then sum
        # the stages and write the block-diagonal bf16 form for the G matmul
        ch = [cpool.tile([128, 2 * KC * DH], BF, name=f"ct_h{i}") for i in range(2)]
        cs = [cpool.tile([128, KC * DH], F32, name=f"ct_s{i}") for i in range(2)]
        for stage in range(2):
            nc.gpsimd.dma_start(
                ch[stage].rearrange("p (r k d) -> p r k d", r=2, d=DH),
                cout[stage].rearrange("r p k d -> p r k d"),
            )
            # on gpsimd: keep the DVE free for the stage-1 softmax tail
            nc.gpsimd.tensor_add(
                cs[stage][:], ch[stage][:, : KC * DH], ch[stage][:, KC * DH :]
            )
        cs_r = [c.rearrange("p (k d) -> p k d", d=DH) for c in cs]
        nc.vector.tensor_add(ctw[:DH, :, 0, :], cs_r[0][:DH], cs_r[1][:DH])
        nc.vector.tensor_add(ctw[DH:, :, 1, :], cs_r[0][DH:], cs_r[1][DH:])

        # ---- phase C: q^T projection (overlaps the collective) ------------
        # Phase B must finish ASAP (the collective chain it feeds is
        # latency-bound), so gate the q matmuls behind the end of B: two
        # groups may start after ct[NT-2] (they cover the last tile's
        # softmax-chain stall), the rest only after the last ct matmul.
        # Without this the scheduler slots q work into every modeled B
        # stall and pushes the collective ~20us later.
        gidx = 0
        for i in range(KC):
            for ntile in range(4):
                first = q_group(i, ntile)
                gate = ct_mm[NT - 2] if gidx < 2 else ct_mm[NT - 1]
                tile.add_dep_helper(
                    first.ins, gate.ins, sync=False, reason="hold q behind phase B"
                )
                gidx += 1

        # ---- phase D: G = blockdiag(context^T) @ w_out^T -------------------
        ps_g = ctx.enter_context(tc.tile_pool(name="ps_g", bufs=2, space="PSUM"))
        ps_y = ctx.enter_context(tc.tile_pool(name="ps_y", bufs=4, space="PSUM"))
        for pr in range(KC):
            lhs = ctw[:, pr].rearrange("p two d -> p (two d)")
            for half in range(2):
                hsl = slice(half * 512, (half + 1) * 512)
                g_ps = ps_g.tile([128, 512], F32, name="g_ps")
                nc.tensor.matmul(
                    g_ps[:], lhsT=lhs, rhs=wout_s[:, pr, hsl], start=True, stop=True
                )
                nc.vector.tensor_copy(g_s[:, pr, hsl], g_ps[:])

        # ---- phase E: yT = G^T @ q^T + b (bias folded into the Scalar copy)
        for f in range(FB):
            fsl = slice(f * 128, (f + 1) * 128)
            for th in range(4):
                tsl = slice(th * 512, (th + 1) * 512)
                y_ps = ps_y.tile([128, 512], F32, name="y_ps")
                for kc in range(KC):
                    nc.tensor.matmul(
                        y_ps[:],
                        lhsT=g_s[:, kc, fsl],
                        rhs=qT_s[:, kc, tsl],
                        start=(kc == 0),
                        stop=(kc == KC - 1),
                    )
                nc.scalar.activation(
                    yT_s[:, f, tsl], y_ps[:], IDENT, bias=biasT_s[:, f : f + 1]
                )
                nc.sync.dma_start(yT[fsl, tsl], yT_s[:, f, tsl])

    return nc


def _prep_inputs(x, w_qkv, w_out, b_out):
    bf16 = ml_dtypes.bfloat16
    x = np.asarray(x, dtype=np.float32)
    w_qkv = np.asarray(w_qkv, dtype=np.float32)
    w_out = np.asarray(w_out, dtype=np.float32)
    b_out = np.asarray(b_out, dtype=np.float32)

    wqT = np.ascontiguousarray(w_qkv[:INNER].T).astype(bf16)  # [D, 512]
    wkvT = np.ascontiguousarray(w_qkv[INNER:].T).astype(bf16)  # [D, 1024]
    woutT = np.ascontiguousarray(w_out.T).astype(bf16)  # [512, D]
    biasT = np.ascontiguousarray(b_out.reshape(FB, 128).T).astype(np.float32)
    xs = x.reshape(B, 2, NTOK, D)
    in_maps = []
    for c in range(NCORES):
        xT = np.ascontiguousarray(xs[c // 2, c % 2].T).astype(bf16)  # [D, NTOK]
        in_maps.append(
            {"xT": xT, "wqT": wqT, "wkvT": wkvT, "woutT": woutT, "biasT": biasT}
        )
    return in_maps


def kernel(x, w_qkv, w_out, b_out):
    global _BUILT
    _install_compile_patch()
    if os.environ.get("BASS_TRACE"):
        _install_ntff_hook()
    from concourse.bass_utils import run_bass_kernel_spmd

    if _BUILT is None:
        _BUILT = build_kernel()
    nc = _BUILT
    in_maps = _prep_inputs(x, w_qkv, w_out, b_out)
    res = run_bass_kernel_spmd(nc, in_maps, core_ids=list(range(NCORES)))
    LAST_RESULT["exec_time_ns"] = res.exec_time_ns
    LAST_RESULT["profile_json"] = res.profile_json
    out = np.empty((B, 2, NTOK, D), dtype=np.float32)
    for c in range(NCORES):
        out[c // 2, c % 2] = np.asarray(res.results[c]["yT"]).T.astype(np.float32)
    return out.reshape(B, SEQ, D)
